# revision 1
# baseline (speedup 1.0000x reference)
"""DiT attention block on 8 Trainium2 NeuronCores.

Reference computation (fp32):
    qkv = x @ Wqkv + b            (b=2, n=2048, din=1024, 3*1024)
    q, k = RMSNorm_full_dim(q|k) * scale  (norm over all 1024 channels)
    RoPE (rotary_dim=64) per 64-dim head, 16 heads
    attn = softmax(q k^T / 8) v ;  out = attn @ Wout + bout

Sharding: 8 cores = 2 batches x 4 head-groups (4 heads / 256 features each).
Two SPMD launches:
  L1: per-core slice of the qkv projection (transposed layout) + partial
      sum-of-squares for the full-dim RMSNorm (host combines: tiny rsqrt).
  L2: norm+rope (DVE), S^T = khat^T qhat (PE, row-tiled head pairs),
      exp (ACT, straight from PSUM), O^T|den = [V|1]^T P (PE, M=65),
      normalize via DVE reciprocal + gpsimd partition_broadcast,
      out-projection partial products.
Host: slices/transposes inputs, rsqrt between launches, sums the 4 partial
projections per batch and adds the (host-folded) v-bias/out-bias term.

All matmuls run as float32r (TF32-like, ~1e-4 rel err, full PE rate).
Column-tiled (tile_position[1] != 0) matmuls are illegal for fp32r, so the
softmax denominator rides along as a 65th column of V instead.
"""

import os
import sys

for _p in ("/opt/trn_rl_repo", "/root/.axon_site/_ro/trn_rl_repo"):
    if os.path.isdir(_p) and _p not in sys.path:
        sys.path.append(_p)

import numpy as np

import concourse.bass as bass  # noqa: E402,F401
import concourse.mybir as mybir  # noqa: E402
import concourse.tile as tile  # noqa: E402
from concourse import bacc  # noqa: E402
from concourse.bass_utils import run_bass_kernel_spmd  # noqa: E402

FP32 = mybir.dt.float32
FP32R = mybir.dt.float32r
AF = mybir.ActivationFunctionType

B = 2
N = 2048
DIN = 1024
DQ = 1024
H = 16
DH = 64
NCORES = 8
NGROUP = 4          # head-groups per batch
GF = 256            # features per core (4 heads)
P = 128
EPS = 1e-6
ROPE_BASE = 10000.0

LAST_EXEC_NS = {}   # filled when KERNEL_TRACE=1
LAST_RESULTS = {}   # BassKernelResults per launch when KERNEL_TRACE=1

_cache = {}


# ----------------------------------------------------------------- launch 1

def _build_l1():
    nc = bacc.Bacc("TRN2", target_bir_lowering=False, debug=False,
                   num_devices=NCORES)
    xT = nc.dram_tensor("xT", [DIN, N], FP32R, kind="ExternalInput")
    wcat = nc.dram_tensor("wcat", [DIN, 3 * GF], FP32R, kind="ExternalInput")
    bqk = nc.dram_tensor("bqk", [P, 4], FP32, kind="ExternalInput")
    invs = nc.dram_tensor("invs", [P, P], FP32R, kind="ExternalInput")
    qT_o = nc.dram_tensor("qT", [GF, N], FP32, kind="ExternalOutput")
    kT_o = nc.dram_tensor("kT", [GF, N], FP32, kind="ExternalOutput")
    v_o = nc.dram_tensor("v", [N, GF], FP32, kind="ExternalOutput")
    ssq_o = nc.dram_tensor("ssq", [2, N], FP32, kind="ExternalOutput")

    KT = DIN // P  # 8 contraction tiles
    NB = N // 512  # 4 column blocks

    with tile.TileContext(nc) as tc:
        with (
            tc.tile_pool(name="xw", bufs=1) as xw,
            tc.tile_pool(name="io", bufs=2) as io,
            tc.tile_pool(name="sqp", bufs=2) as sqp,
            tc.tile_pool(name="stgp", bufs=4) as stgp,
            tc.tile_pool(name="ps", bufs=2, space="PSUM") as ps,
            tc.tile_pool(name="pssq", bufs=4, space="PSUM") as pssq,
        ):
            xt = []
            wt = []
            for kt in range(KT):
                t = xw.tile([P, N], FP32R, tag=f"xt{kt}")
                nc.sync.dma_start(t[:], xT[kt * P:(kt + 1) * P, :])
                xt.append(t)
                w = xw.tile([P, 3 * GF], FP32R, tag=f"wt{kt}")
                nc.sync.dma_start(w[:], wcat[kt * P:(kt + 1) * P, :])
                wt.append(w)
            bias = xw.tile([P, 4], FP32, tag="bias")
            nc.sync.dma_start(bias[:], bqk[:, :])
            winv = xw.tile([P, P], FP32R, tag="winv")
            nc.sync.dma_start(winv[:], invs[:, :])

            # q', k' in transposed layout, with bias, + partial ssq
            for t_idx, (col0, out_dram) in enumerate(((0, qT_o), (GF, kT_o))):
                bigs = []
                for mt in range(GF // P):
                    big = io.tile([P, N], FP32, tag="qk")
                    for nb in range(NB):
                        acc = ps.tile([P, 512], FP32, tag="acc")
                        for kt in range(KT):
                            nc.tensor.matmul(
                                acc[:],
                                wt[kt][:, col0 + mt * P: col0 + (mt + 1) * P],
                                xt[kt][:, nb * 512:(nb + 1) * 512],
                                start=(kt == 0),
                                stop=(kt == KT - 1),
                            )
                        nc.scalar.activation(
                            big[:, nb * 512:(nb + 1) * 512], acc[:],
                            AF.Identity,
                            bias=bias[:, 2 * t_idx + mt: 2 * t_idx + mt + 1],
                        )
                    nc.sync.dma_start(out_dram[mt * P:(mt + 1) * P, :], big[:])
                    bigs.append(big)

                # partial weighted sum-of-squares for this tensor:
                # 32 identical output rows (all-equal lhsT columns), M=32 at
                # tile (0,0) -- col-tiled fp32r matmuls are illegal.
                for nb in range(NB):
                    sp = pssq.tile([32, 512], FP32, tag="ssq",
                                   name=f"ssq{t_idx}_{nb}")
                    for mt in range(GF // P):
                        sq = sqp.tile([P, 512], FP32R, tag="sq")
                        nc.vector.tensor_tensor(
                            sq[:],
                            bigs[mt][:, nb * 512:(nb + 1) * 512],
                            bigs[mt][:, nb * 512:(nb + 1) * 512],
                            mybir.AluOpType.mult)
                        nc.tensor.matmul(
                            sp[:],
                            winv[:, 32 * (2 * t_idx + mt):
                                 32 * (2 * t_idx + mt + 1)],
                            sq[:],
                            start=(mt == 0),
                            stop=(mt == GF // P - 1),
                        )
                    stg = stgp.tile([1, 512], FP32, tag="stg",
                                    name=f"stg{t_idx}_{nb}")
                    nc.vector.tensor_copy(stg[:], sp[0:1, :])
                    nc.sync.dma_start(
                        ssq_o[t_idx:t_idx + 1, nb * 512:(nb + 1) * 512],
                        stg[:])

            # v in natural (token, feature) layout, no bias
            for tt in range(N // P):
                acc = ps.tile([P, GF], FP32, tag="vacc")
                for kt in range(KT):
                    nc.tensor.matmul(
                        acc[:],
                        xt[kt][:, tt * P:(tt + 1) * P],
                        wt[kt][:, 2 * GF:3 * GF],
                        start=(kt == 0),
                        stop=(kt == KT - 1),
                    )
                vsb = io.tile([P, GF], FP32, tag="v")
                nc.vector.tensor_copy(vsb[:], acc[:])
                nc.sync.dma_start(v_o[tt * P:(tt + 1) * P, :], vsb[:])

    nc.compile()
    return nc


# ----------------------------------------------------------------- launch 2

def _build_l2():
    nc = bacc.Bacc("TRN2", target_bir_lowering=False, debug=False,
                   num_devices=NCORES)
    qT = nc.dram_tensor("qT", [GF, N], FP32, kind="ExternalInput")
    kT = nc.dram_tensor("kT", [GF, N], FP32, kind="ExternalInput")
    # v with a ones-column appended per head: [v_h (64) | 1] x 4 heads
    v_i = nc.dram_tensor("v", [N, 4 * 65], FP32R, kind="ExternalInput")
    cosq_i = nc.dram_tensor("cosq", [P, N], FP32, kind="ExternalInput")
    sinq_i = nc.dram_tensor("sinq", [P, N], FP32, kind="ExternalInput")
    cosk_i = nc.dram_tensor("cosk", [P, N], FP32, kind="ExternalInput")
    sink_i = nc.dram_tensor("sink", [P, N], FP32, kind="ExternalInput")
    wout_i = nc.dram_tensor("wout", [GF, DIN], FP32R, kind="ExternalInput")
    part_o = nc.dram_tensor("part", [2, N, DIN], FP32, kind="ExternalOutput")

    IBW = 512        # query-block width
    NIB = N // IBW   # 4 query blocks
    NJT = N // P     # 16 key tiles

    with tile.TileContext(nc) as tc:
        with (
            tc.tile_pool(name="cst", bufs=1) as cst,
            tc.tile_pool(name="scr", bufs=2) as scr,
            tc.tile_pool(name="hat", bufs=1) as hatp,
            tc.tile_pool(name="ptp", bufs=4) as ptp,
            tc.tile_pool(name="obig", bufs=1) as obigp,
            tc.tile_pool(name="onrm", bufs=2) as onrm,
            tc.tile_pool(name="outp", bufs=2) as outp,
            tc.tile_pool(name="tiny", bufs=2) as tiny,
            tc.tile_pool(name="psS", bufs=2, space="PSUM") as psS,
            tc.tile_pool(name="psO", bufs=4, space="PSUM") as psO,
        ):
            # ---- phase A: normalize + rope -> qhat/khat (fp32r) ----
            # The RMSNorm factor is folded into per-tensor cos/sin tables on
            # the host, so each tile chain is only 3 DVE passes.  Pair-0
            # tiles (k0, q0) first so pair-0 attention starts early.
            tabs = {}
            for name, ci, si in (("k", cosk_i, sink_i), ("q", cosq_i, sinq_i)):
                cr = cst.tile([P, N], FP32, tag=f"cos_{name}")
                nc.sync.dma_start(cr[:], ci[:, :])
                sr = cst.tile([P, N], FP32, tag=f"sin_{name}")
                nc.sync.dma_start(sr[:], si[:, :])
                tabs[name] = (cr, sr)
            hats = {}
            for name, src_dram, mt in (("k", kT, 0), ("q", qT, 0),
                                       ("k", kT, 1), ("q", qT, 1)):
                cr, sr = tabs[name]
                t1 = scr.tile([P, N], FP32, tag="t1")
                nc.sync.dma_start(t1[:], src_dram[mt * P:(mt + 1) * P, :])
                sh = scr.tile([P, N], FP32, tag="sh")
                for blk in range(4):
                    srcb = blk ^ 1
                    nc.sync.dma_start(sh[blk * 32:(blk + 1) * 32, :],
                                      t1[srcb * 32:(srcb + 1) * 32, :])
                t2 = scr.tile([P, N], FP32, tag="t2")
                nc.vector.tensor_mul(t2[:], t1[:], cr[:])
                nc.gpsimd.tensor_mul(sh[:], sh[:], sr[:])
                hat = hatp.tile([P, N], FP32R, tag=f"hat_{name}{mt}")
                nc.vector.tensor_add(hat[:], t2[:], sh[:])
                hats[(name, mt)] = hat

            vbig = []
            for vb in range(NJT // 4):
                t = cst.tile([P, 4 * 4 * 65], FP32R, tag=f"v{vb}")
                nc.sync.dma_start(
                    t[:].rearrange("p (a d) -> p a d", a=4),
                    v_i[vb * 4 * P:(vb + 1) * 4 * P, :].rearrange(
                        "(a p) d -> p a d", p=P))
                vbig.append(t)

            def vt_slice(jt, h):
                return vbig[jt // 4][:, (jt % 4) * 4 * 65 + h * 65:
                                     (jt % 4) * 4 * 65 + (h + 1) * 65]
            wout = []
            for kt in range(2):
                w = cst.tile([P, DIN], FP32R, tag=f"wo{kt}")
                nc.sync.dma_start(w[:], wout_i[kt * P:(kt + 1) * P, :])
                wout.append(w)

            osb_pend = {}

            def emit_proj(pr, tt):
                # one token-tile of the out-projection partial for pair pr;
                # PE is in-order, so these are interleaved into pair-1's
                # (ACT-bound) attention stream. Host adds the two partials.
                # Two token-tiles share one SBUF staging tile and go out in
                # a single rearranged DMA to halve the sync-queue load.
                pss = [psO.tile([P, 512], FP32, tag="O",
                                name=f"pj{pr}_{tt}_{hf}")
                       for hf in range(2)]
                for half in range(2):
                    nc.tensor.matmul(
                        pss[half][:],
                        obig[pr][:, tt * P:(tt + 1) * P],
                        wout[pr][:, half * 512:(half + 1) * 512],
                        start=True, stop=True,
                    )
                if tt % 2 == 0:
                    osb_pend[pr] = outp.tile([P, 2 * DIN], FP32, tag="osb",
                                             name=f"osb{pr}_{tt}")
                osb = osb_pend[pr]
                base = (tt % 2) * DIN
                nc.vector.tensor_copy(osb[:, base:base + 512], pss[0][:])
                nc.vector.tensor_copy(osb[:, base + 512:base + 1024],
                                      pss[1][:])
                if tt % 2 == 1:
                    nc.sync.dma_start(
                        part_o[pr, (tt - 1) * P:(tt + 1) * P, :].rearrange(
                            "(a p) d -> p a d", p=P),
                        osb[:].rearrange("p (a d) -> p a d", a=2))

            # ---- phase B: attention ----
            # obig[pair]: normalized O^T for heads (2*pair, 2*pair+1).
            # Pair-outer loop: each head's S slice owns a full PSUM bank
            # (two start=True groups must never share a bank), and the
            # two-bank S tile double-buffers against the single exp call.
            obig = [obigp.tile([P, N], FP32R, tag=f"obig{pr}", name=f"ob{pr}")
                    for pr in range(2)]
            def finish_block(pr, ib, o_ps):
                # normalize: row 64 of each o_ps is the denominator.
                # The cross-partition DMA runs BEFORE the (slow) reciprocal
                # so the in-order sync queue only ever waits on a cheap DVE
                # copy; the reciprocal runs at partition 0 afterwards.
                rrd, rr0, bc = [], [], []
                for sub in range(2):
                    t = tiny.tile([65, IBW], FP32, tag="rrd",
                                  name=f"rrd{pr}_{ib}_{sub}")
                    nc.vector.tensor_copy(t[64:65, :], o_ps[sub][64:65, :])
                    rrd.append(t)
                for sub in range(2):
                    t = tiny.tile([1, IBW], FP32, tag="rr0",
                                  name=f"rr0{pr}_{ib}_{sub}")
                    nc.sync.dma_start(t[:, :], rrd[sub][64:65, :])
                    rr0.append(t)
                for sub in range(2):
                    nc.vector.reciprocal(rr0[sub][:, :], rr0[sub][:, :])
                for sub in range(2):
                    t = tiny.tile([64, IBW], FP32, tag="bc",
                                  name=f"bc{pr}_{ib}_{sub}")
                    nc.gpsimd.partition_broadcast(t[:, :], rr0[sub][:, :])
                    bc.append(t)
                nc.vector.tensor_mul(
                    obig[pr][0:64, ib * IBW:(ib + 1) * IBW],
                    o_ps[0][0:64, :], bc[0][:, :])
                onr = onrm.tile([64, IBW], FP32R, tag="onr")
                nc.vector.tensor_mul(onr[:, :], o_ps[1][0:64, :], bc[1][:, :])
                nc.sync.dma_start(
                    obig[pr][64:128, ib * IBW:(ib + 1) * IBW], onr[:, :])

            # Flattened attention stream over (pair, ib, jt): the S matmul +
            # exp are issued one step ahead of the AV matmul across ALL
            # block boundaries, so the (bottleneck) exp on ACT never waits
            # for the in-order tensor engine.  proj injections for pair 0
            # eat the AV-side slack during pair 1.
            steps = [(pr, ib, jt) for pr in range(2) for ib in range(NIB)
                     for jt in range(NJT)]
            p_sbs = {}
            o_ps_map = {}

            def emit_s(step):
                pr, ib, jt = step
                s_ps = psS.tile([P, 2 * IBW], FP32, tag="S")
                for sub in range(2):
                    nc.tensor.matmul(
                        s_ps[:, sub * IBW:(sub + 1) * IBW],
                        hats[("k", pr)][sub * 64:(sub + 1) * 64,
                                        jt * P:(jt + 1) * P],
                        hats[("q", pr)][sub * 64:(sub + 1) * 64,
                                        ib * IBW:(ib + 1) * IBW],
                        start=True, stop=True,
                        tile_position=(64 * sub, 0),
                    )
                p_sb = ptp.tile([P, 2 * IBW], FP32R, tag="P",
                                name=f"p{pr}_{ib}_{jt}")
                nc.scalar.activation(p_sb[:, :], s_ps[:, :],
                                     AF.Exp, scale=0.125)
                p_sbs[step] = p_sb

            emit_s(steps[0])
            for si, step in enumerate(steps):
                pr, ib, jt = step
                if si + 1 < len(steps):
                    emit_s(steps[si + 1])
                if (pr, ib) not in o_ps_map:
                    o_ps_map[(pr, ib)] = [
                        psO.tile([65, IBW], FP32, tag="O",
                                 name=f"o{pr}_{ib}_{s}") for s in range(2)]
                o_ps = o_ps_map[(pr, ib)]
                p_sb = p_sbs.pop(step)
                for sub in range(2):
                    h = 2 * pr + sub
                    nc.tensor.matmul(
                        o_ps[sub][:, :],
                        vt_slice(jt, h),
                        p_sb[:, sub * IBW:(sub + 1) * IBW],
                        start=(jt == 0), stop=(jt == NJT - 1),
                    )
                if pr == 1 and jt % 4 == 2:
                    emit_proj(0, ib * 4 + jt // 4)
                if jt == NJT - 1:
                    finish_block(pr, ib, o_ps)

            for tt in range(N // P):
                emit_proj(1, tt)

    nc.compile()
    return nc


# ------------------------------------------------------------------- driver

def _rope_tables():
    half = DH // 2
    inv_freq = 1.0 / (ROPE_BASE ** (np.arange(half, dtype=np.float64) * 2.0
                                    / DH))
    freqs = np.arange(N, dtype=np.float64)[:, None] * inv_freq[None, :]
    cos = np.cos(freqs).T          # (32, N)
    sin = np.sin(freqs).T
    cos64 = np.concatenate([cos, cos], 0)            # (64, N)
    sin64 = np.concatenate([-sin, sin], 0)           # signed for rotate_half
    cos_t = np.ascontiguousarray(
        np.concatenate([cos64, cos64], 0).astype(np.float32))  # (128, N)
    sin_t = np.ascontiguousarray(
        np.concatenate([sin64, sin64], 0).astype(np.float32))
    return cos_t, sin_t


def kernel(input, w_qkv, b_qkv, q_scale, k_scale, w_out, b_out):
    trace = bool(os.environ.get("KERNEL_TRACE"))
    if "l1" not in _cache:
        _cache["l1"] = _build_l1()
    if "l2" not in _cache:
        _cache["l2"] = _build_l2()

    x = np.asarray(input, dtype=np.float32)
    w_qkv = np.asarray(w_qkv, dtype=np.float32)
    b_qkv = np.asarray(b_qkv, dtype=np.float32)
    qs = np.asarray(q_scale, dtype=np.float32)
    ks = np.asarray(k_scale, dtype=np.float32)
    w_out = np.asarray(w_out, dtype=np.float32)
    b_out = np.asarray(b_out, dtype=np.float32)

    wq = w_qkv[:, :DQ] * qs[None, :]
    wk = w_qkv[:, DQ:2 * DQ] * ks[None, :]
    wv = w_qkv[:, 2 * DQ:]
    bq = b_qkv[:DQ] * qs
    bk = b_qkv[DQ:2 * DQ] * ks
    bv = b_qkv[2 * DQ:]

    xT = [np.ascontiguousarray(x[b].T) for b in range(B)]

    def col4(vec256_a, vec256_b):
        # -> (128, 4): [a_mt0 | a_mt1 | b_mt0 | b_mt1]
        return np.ascontiguousarray(np.stack(
            [vec256_a[:P], vec256_a[P:], vec256_b[:P], vec256_b[P:]],
            axis=1).astype(np.float32))

    in1 = []
    for c in range(NCORES):
        b, g = divmod(c, NGROUP)
        sl = slice(g * GF, (g + 1) * GF)
        wcat = np.ascontiguousarray(
            np.concatenate([wq[:, sl], wk[:, sl], wv[:, sl]], axis=1))
        in1.append({
            "xT": xT[b],
            "wcat": wcat,
            "bqk": col4(bq[sl], bk[sl]),
            "invs": np.ascontiguousarray(np.repeat(
                col4(1.0 / np.square(qs[sl]), 1.0 / np.square(ks[sl])),
                32, axis=1)),
        })

    r1 = run_bass_kernel_spmd(_cache["l1"], in1,
                              core_ids=list(range(NCORES)), trace=trace)
    if trace:
        LAST_EXEC_NS["l1"] = r1.exec_time_ns
        LAST_RESULTS["l1"] = r1

    # host: combine partial ssq -> rsqrt factors folded into rope tables
    cos_t, sin_t = _rope_tables()
    tabs = {}
    for b in range(B):
        sq_q = np.zeros(N, np.float64)
        sq_k = np.zeros(N, np.float64)
        for g in range(NGROUP):
            ssq = r1.results[NGROUP * b + g]["ssq"].astype(np.float64)
            sq_q += ssq[0]
            sq_k += ssq[1]
        r_q = (1.0 / np.sqrt(sq_q / DQ + EPS)).astype(np.float32)
        r_k = (1.0 / np.sqrt(sq_k / DQ + EPS)).astype(np.float32)
        tabs[b] = {
            "cosq": np.ascontiguousarray(cos_t * r_q[None, :]),
            "sinq": np.ascontiguousarray(sin_t * r_q[None, :]),
            "cosk": np.ascontiguousarray(cos_t * r_k[None, :]),
            "sink": np.ascontiguousarray(sin_t * r_k[None, :]),
        }

    in2 = []
    for c in range(NCORES):
        b, g = divmod(c, NGROUP)
        sl = slice(g * GF, (g + 1) * GF)
        v = r1.results[c]["v"]                       # (N, 256)
        v65 = np.ones((N, 4 * 65), np.float32)
        for h in range(4):
            v65[:, h * 65:h * 65 + 64] = v[:, h * 64:(h + 1) * 64]
        in2.append({
            "qT": r1.results[c]["qT"],
            "kT": r1.results[c]["kT"],
            "v": np.ascontiguousarray(v65),
            "wout": np.ascontiguousarray(w_out[sl, :]),
            **tabs[b],
        })

    r2 = run_bass_kernel_spmd(_cache["l2"], in2,
                              core_ids=list(range(NCORES)), trace=trace)
    if trace:
        LAST_EXEC_NS["l2"] = r2.exec_time_ns
        LAST_RESULTS["l2"] = r2

    base = (bv.astype(np.float64) @ w_out.astype(np.float64)
            + b_out.astype(np.float64))
    out = np.zeros((B, N, DIN), np.float32)
    for b in range(B):
        acc = np.zeros((N, DIN), np.float64)
        for g in range(NGROUP):
            p = r2.results[NGROUP * b + g]["part"].astype(np.float64)
            acc += p[0]
            acc += p[1]
        out[b] = (acc + base[None, :]).astype(np.float32)
    return out



# revision 9
# speedup vs baseline: 1.0974x; 1.0974x over previous
"""DiT attention block on 8 Trainium2 NeuronCores — fused single launch.

Reference computation (fp32):
    qkv = x @ Wqkv + b            (b=2, n=2048, din=1024, 3*1024)
    q, k = RMSNorm_full_dim(q|k) * scale  (norm over all 1024 channels)
    RoPE (rotary_dim=64) per 64-dim head, 16 heads
    attn = softmax(q k^T / 8) v ;  out = attn @ Wout + bout

Sharding: 8 cores = 2 batches x 4 head-groups (4 heads / 256 features each).
ONE SPMD launch per core:
  k-proj -> ssq_k partial -> AllReduce([0-3],[4-7]) ; q-proj -> ssq_q ->
  AllReduce ; v-proj (PE stays dense) while rsqrt + rope run on ACT/DVE/
  GpSimd; then the flattened attention stream (S matmul -> exp on ACT ->
  AV matmul with softmax denominator as a 65th V column), out-projection
  partials injected into the stream as each 512-query block normalizes.
Host: preps transposed/bf16 inputs, sums the 8x2 projection partials and
adds the (host-folded) v-bias/out-bias term.

Precision: matmul inputs bf16 except qhat/khat (fp32r) so the softmax
logits stay accurate; PSUM accumulation fp32 everywhere; the RMSNorm
rsqrt and softmax reciprocal run at fp32 (reciprocal_approx_fast).
"""

import os
import sys

for _p in ("/opt/trn_rl_repo", "/root/.axon_site/_ro/trn_rl_repo"):
    if os.path.isdir(_p) and _p not in sys.path:
        sys.path.append(_p)

import numpy as np
import ml_dtypes

import concourse.bass as bass  # noqa: E402,F401
import concourse.mybir as mybir  # noqa: E402
import concourse.tile as tile  # noqa: E402
from concourse import bacc  # noqa: E402
from concourse.bass_utils import run_bass_kernel_spmd  # noqa: E402

FP32 = mybir.dt.float32
FP32R = mybir.dt.float32r
BF16 = mybir.dt.bfloat16
AF = mybir.ActivationFunctionType
BF16NP = ml_dtypes.bfloat16

B = 2
N = 2048
DIN = 1024
DQ = 1024
H = 16
DH = 64
NCORES = 8
NGROUP = 4          # head-groups per batch
GF = 256            # features per core (4 heads)
P = 128
EPS = 1e-6
ROPE_BASE = 10000.0

IBW = 512           # query-block width
NIB = N // IBW      # 4 query blocks
NJT = N // P        # 16 key tiles
KT = DIN // P       # 8 contraction tiles

LAST_EXEC_NS = {}   # filled when KERNEL_TRACE=1
LAST_RESULTS = {}

_cache = {}


def _build_fused():
    nc = bacc.Bacc("TRN2", target_bir_lowering=False, debug=False,
                   num_devices=NCORES)
    xT_i = nc.dram_tensor("xT", [DIN, N], BF16, kind="ExternalInput")
    wcat_i = nc.dram_tensor("wcat", [DIN, 3 * GF], BF16, kind="ExternalInput")
    bqk_i = nc.dram_tensor("bqk", [P, 4], FP32, kind="ExternalInput")
    winv_i = nc.dram_tensor("winv", [P, P], BF16, kind="ExternalInput")
    cos_i = nc.dram_tensor("cos_t", [P, N], BF16, kind="ExternalInput")
    sin_i = nc.dram_tensor("sin_t", [P, N], BF16, kind="ExternalInput")
    wout_i = nc.dram_tensor("wout", [GF, DIN], BF16, kind="ExternalInput")
    part_o = nc.dram_tensor("part", [2, N, DIN], BF16, kind="ExternalOutput")

    groups = [[0, 1, 2, 3], [4, 5, 6, 7]]

    with tile.TileContext(nc) as tc:
        with (
            tc.tile_pool(name="cst", bufs=1) as cst,
            tc.tile_pool(name="hat", bufs=1) as hatp,
            tc.tile_pool(name="obig", bufs=1) as obigp,
            tc.tile_pool(name="dram", bufs=1, space="DRAM") as dram,
        ):
            # ---------------- constant loads ----------------
            xt, wt = [], []
            for kt in range(KT):
                t = cst.tile([P, N], BF16, tag=f"xt{kt}", name=f"xt{kt}")
                nc.sync.dma_start(t[:], xT_i[kt * P:(kt + 1) * P, :])
                xt.append(t)
                w = cst.tile([P, 3 * GF], BF16, tag=f"wt{kt}", name=f"wt{kt}")
                nc.sync.dma_start(w[:], wcat_i[kt * P:(kt + 1) * P, :])
                wt.append(w)
            bias = cst.tile([P, 4], FP32, tag="bias")
            nc.sync.dma_start(bias[:], bqk_i[:, :])
            winv = cst.tile([P, P], BF16, tag="winv")
            nc.sync.dma_start(winv[:], winv_i[:, :])
            cos_sb = cst.tile([P, N], BF16, tag="cos_sb")
            nc.sync.dma_start(cos_sb[:], cos_i[:, :])
            sin_sb = cst.tile([P, N], BF16, tag="sin_sb")
            nc.sync.dma_start(sin_sb[:], sin_i[:, :])
            wout = []
            for kt in range(2):
                w = cst.tile([P, DIN], BF16, tag=f"wo{kt}", name=f"wo{kt}")
                nc.sync.dma_start(w[:], wout_i[kt * P:(kt + 1) * P, :])
                wout.append(w)
            # packed V tiles: per vb, 4 token-subtiles x 4 heads x 65 cols
            # (64 v-features + a ones column for the softmax denominator)
            vbig = [cst.tile([P, 4 * 4 * 65], BF16, tag=f"v{vb}",
                             name=f"v{vb}")
                    for vb in range(NJT // 4)]
            for vb in range(NJT // 4):
                ones_ap = vbig[vb][:].rearrange(
                    "p (g c) -> p g c", g=16)[:, :, 64:65]
                nc.vector.memset(ones_ap, 1.0)

            def vt_slice(jt, h):
                base = (jt % 4) * 4 * 65 + h * 65
                return vbig[jt // 4][:, base:base + 65]

            # CC bounce buffers (DRAM)
            ssq_in = [dram.tile([1, N], FP32, tag=f"cci{t}", name=f"cci{t}")
                      for t in range(2)]
            ssq_out = [dram.tile([1, N], FP32, tag=f"cco{t}", name=f"cco{t}")
                       for t in range(2)]

            obig = [obigp.tile([P, N], BF16, tag=f"obig{pr}", name=f"ob{pr}")
                    for pr in range(2)]

            # ---------------- phase 1: qkv projections + ssq ----------------
            prime = {}
            with (
                tc.tile_pool(name="prm", bufs=1) as prm,
                tc.tile_pool(name="sqp", bufs=2) as sqp,
                tc.tile_pool(name="stgp", bufs=2) as stgp,
                tc.tile_pool(name="ps", bufs=2, space="PSUM") as ps,
                tc.tile_pool(name="pssq", bufs=2, space="PSUM") as pssq,
                tc.tile_pool(name="psv", bufs=2, space="PSUM") as psv,
            ):
                # k first so its AllReduce is in flight the longest
                for name, t_idx in (("k", 1), ("q", 0)):
                    col0 = GF if t_idx == 1 else 0
                    for mt in range(2):
                        big = prm.tile([P, N], BF16, tag=f"{name}{mt}",
                                       name=f"{name}{mt}")
                        prime[(name, mt)] = big
                        for nb in range(4):
                            acc = ps.tile([P, 512], FP32, tag="acc")
                            for kt in range(KT):
                                nc.tensor.matmul(
                                    acc[:],
                                    wt[kt][:, col0 + mt * P:
                                           col0 + (mt + 1) * P],
                                    xt[kt][:, nb * 512:(nb + 1) * 512],
                                    start=(kt == 0),
                                    stop=(kt == KT - 1),
                                )
                            nc.scalar.activation(
                                big[:, nb * 512:(nb + 1) * 512], acc[:],
                                AF.Identity,
                                bias=bias[:, 2 * t_idx + mt:
                                          2 * t_idx + mt + 1],
                            )
                    # weighted sum-of-squares partial -> DRAM bounce
                    for nb in range(4):
                        sp = pssq.tile([32, 512], FP32, tag="ssq",
                                       name=f"ssq{t_idx}_{nb}")
                        for mt in range(2):
                            sq = sqp.tile([P, 512], BF16, tag="sq")
                            nc.vector.tensor_mul(
                                sq[:],
                                prime[(name, mt)][:, nb * 512:(nb + 1) * 512],
                                prime[(name, mt)][:, nb * 512:(nb + 1) * 512])
                            nc.tensor.matmul(
                                sp[:],
                                winv[:, 32 * (2 * t_idx + mt):
                                     32 * (2 * t_idx + mt + 1)],
                                sq[:],
                                start=(mt == 0),
                                stop=(mt == 1),
                            )
                        stg = stgp.tile([1, 512], FP32, tag="stg",
                                        name=f"stg{t_idx}_{nb}")
                        nc.vector.tensor_copy(stg[:], sp[0:1, :])
                        nc.gpsimd.dma_start(
                            ssq_in[t_idx][0:1, nb * 512:(nb + 1) * 512],
                            stg[:])
                    nc.gpsimd.collective_compute(
                        "AllReduce",
                        mybir.AluOpType.add,
                        replica_groups=groups,
                        ins=[ssq_in[t_idx][:]],
                        outs=[ssq_out[t_idx][:]],
                    )

                # ---- rope tables + hats (issued before v so DVE/ACT work
                # overlaps the v matmuls; PE order is unaffected) ----
                with (
                    tc.tile_pool(name="rp", bufs=1) as rp,
                    tc.tile_pool(name="scr", bufs=1) as scr,
                ):
                    eff = {}
                    for name, t_idx in (("k", 1), ("q", 0)):
                        st = rp.tile([1, N], FP32, tag="st", bufs=2,
                                     name=f"st{name}")
                        nc.gpsimd.dma_start(st[:], ssq_out[t_idx][:])
                        nc.vector.tensor_scalar(
                            st[:], st[:], 1.0 / DQ, EPS,
                            mybir.AluOpType.mult, mybir.AluOpType.add)
                        nc.scalar.activation(st[:], st[:], AF.Sqrt)
                        nc.vector.reciprocal_approx_fast(out=st[:], in_=st[:])
                        bcr = rp.tile([P, N], FP32, tag="bcr", bufs=2,
                                      name=f"bcr{name}")
                        nc.gpsimd.partition_broadcast(bcr[:, :], st[:, :])
                        ce = rp.tile([P, N], BF16, tag="ce", bufs=2,
                                     name=f"ce{name}")
                        nc.vector.tensor_mul(ce[:], cos_sb[:], bcr[:])
                        se = rp.tile([P, N], BF16, tag="se", bufs=2,
                                     name=f"se{name}")
                        nc.vector.tensor_mul(se[:], sin_sb[:], bcr[:])
                        eff[name] = (ce, se)

                    hats = {}
                    for name, mt in (("k", 0), ("k", 1), ("q", 0), ("q", 1)):
                        big = prime[(name, mt)]
                        ce, se = eff[name]
                        sh = scr.tile([P, N], BF16, tag="sh", bufs=2,
                                      name=f"sh{name}{mt}")
                        for blk in range(4):
                            srcb = blk ^ 1
                            nc.sync.dma_start(
                                sh[blk * 32:(blk + 1) * 32, :],
                                big[srcb * 32:(srcb + 1) * 32, :])
                        # in place: prime's other readers (ssq squares and
                        # the shift DMAs above) are already issued
                        nc.vector.tensor_mul(big[:], big[:], ce[:])
                        nc.gpsimd.tensor_mul(sh[:], sh[:], se[:])
                        hat = hatp.tile([P, N], FP32R, tag=f"hat_{name}{mt}",
                                        name=f"hat_{name}{mt}")
                        nc.vector.tensor_add(hat[:], big[:], sh[:])
                        hats[(name, mt)] = hat

                    # ---- v projection (PE keeps running while rope DVE
                    # chain executes) ----
                    for tt in range(NJT):
                        acc = psv.tile([P, GF], FP32, tag="vacc")
                        for kt in range(KT):
                            nc.tensor.matmul(
                                acc[:],
                                xt[kt][:, tt * P:(tt + 1) * P],
                                wt[kt][:, 2 * GF:3 * GF],
                                start=(kt == 0),
                                stop=(kt == KT - 1),
                            )
                        out_ap = vbig[tt // 4][:].rearrange(
                            "p (g c) -> p g c",
                            g=16)[:, 4 * (tt % 4):4 * (tt % 4) + 4, 0:64]
                        nc.scalar.activation(
                            out_ap,
                            acc[:].rearrange("p (h d) -> p h d", h=4),
                            AF.Identity)

            # ---------------- phase 2: attention ----------------
            with (
                tc.tile_pool(name="ptp", bufs=4) as ptp,
                tc.tile_pool(name="onrm", bufs=2) as onrm,
                tc.tile_pool(name="outp", bufs=2) as outp,
                tc.tile_pool(name="tiny", bufs=2) as tiny,
                tc.tile_pool(name="psS", bufs=2, space="PSUM") as psS,
                tc.tile_pool(name="psO", bufs=4, space="PSUM") as psO,
            ):
                osb_pend = {}
                proj_pending = []

                def emit_proj_unit(pr, tt, half):
                    pss = psO.tile([P, 512], FP32, tag="O",
                                   name=f"pj{pr}_{tt}_{half}")
                    nc.tensor.matmul(
                        pss[:],
                        obig[pr][:, tt * P:(tt + 1) * P],
                        wout[pr][:, half * 512:(half + 1) * 512],
                        start=True, stop=True,
                    )
                    if (tt % 2, half) == (0, 0):
                        osb_pend[pr] = outp.tile([P, 2 * DIN], BF16,
                                                 tag="osb",
                                                 name=f"osb{pr}_{tt}")
                    osb = osb_pend[pr]
                    base = (tt % 2) * DIN + half * 512
                    nc.vector.tensor_copy(osb[:, base:base + 512], pss[:])
                    if (tt % 2, half) == (1, 1):
                        nc.sync.dma_start(
                            part_o[pr, (tt - 1) * P:(tt + 1) * P, :]
                            .rearrange("(a p) d -> p a d", p=P),
                            osb[:].rearrange("p (a d) -> p a d", a=2))

                def finish_block(pr, ib, o_ps):
                    # row 64 of each o_ps is the softmax denominator.
                    rrd, rr0, bc = [], [], []
                    for sub in range(2):
                        t = tiny.tile([65, IBW], FP32, tag="rrd",
                                      name=f"rrd{pr}_{ib}_{sub}")
                        nc.vector.tensor_copy(t[64:65, :],
                                              o_ps[sub][64:65, :])
                        rrd.append(t)
                    for sub in range(2):
                        t = tiny.tile([1, IBW], FP32, tag="rr0",
                                      name=f"rr0{pr}_{ib}_{sub}")
                        nc.sync.dma_start(t[:, :], rrd[sub][64:65, :])
                        rr0.append(t)
                    for sub in range(2):
                        nc.vector.reciprocal_approx_fast(
                            out=rr0[sub][:, :], in_=rr0[sub][:, :])
                    for sub in range(2):
                        t = tiny.tile([64, IBW], FP32, tag="bc",
                                      name=f"bc{pr}_{ib}_{sub}")
                        nc.gpsimd.partition_broadcast(t[:, :], rr0[sub][:, :])
                        bc.append(t)
                    nc.vector.tensor_mul(
                        obig[pr][0:64, ib * IBW:(ib + 1) * IBW],
                        o_ps[0][0:64, :], bc[0][:, :])
                    onr = onrm.tile([64, IBW], BF16, tag="onr")
                    nc.vector.tensor_mul(onr[:, :], o_ps[1][0:64, :],
                                         bc[1][:, :])
                    nc.sync.dma_start(
                        obig[pr][64:128, ib * IBW:(ib + 1) * IBW], onr[:, :])
                    # queue this block's out-projection into the stream
                    for tt in range(ib * 4, ib * 4 + 4):
                        for half in range(2):
                            proj_pending.append((pr, tt, half))

                steps = [(pr, ib, jt) for pr in range(2) for ib in range(NIB)
                         for jt in range(NJT)]
                p_sbs = {}
                o_ps_map = {}

                def emit_s(step):
                    pr, ib, jt = step
                    s_ps = psS.tile([P, 2 * IBW], FP32, tag="S")
                    for sub in range(2):
                        nc.tensor.matmul(
                            s_ps[:, sub * IBW:(sub + 1) * IBW],
                            hats[("k", pr)][sub * 64:(sub + 1) * 64,
                                            jt * P:(jt + 1) * P],
                            hats[("q", pr)][sub * 64:(sub + 1) * 64,
                                            ib * IBW:(ib + 1) * IBW],
                            start=True, stop=True,
                            tile_position=(64 * sub, 0),
                        )
                    p_sb = ptp.tile([P, 2 * IBW], BF16, tag="P",
                                    name=f"p{pr}_{ib}_{jt}")
                    nc.scalar.activation(p_sb[:, :], s_ps[:, :],
                                         AF.Exp, scale=0.125)
                    p_sbs[step] = p_sb

                emit_s(steps[0])
                for si, step in enumerate(steps):
                    pr, ib, jt = step
                    if si + 1 < len(steps):
                        emit_s(steps[si + 1])
                    if (pr, ib) not in o_ps_map:
                        o_ps_map[(pr, ib)] = [
                            psO.tile([65, IBW], FP32, tag="O",
                                     name=f"o{pr}_{ib}_{s}")
                            for s in range(2)]
                    o_ps = o_ps_map[(pr, ib)]
                    p_sb = p_sbs.pop(step)
                    for sub in range(2):
                        h = 2 * pr + sub
                        nc.tensor.matmul(
                            o_ps[sub][:, :],
                            vt_slice(jt, h),
                            p_sb[:, sub * IBW:(sub + 1) * IBW],
                            start=(jt == 0), stop=(jt == NJT - 1),
                        )
                    if proj_pending and si % 2 == 1:
                        emit_proj_unit(*proj_pending.pop(0))
                    if jt == NJT - 1:
                        finish_block(pr, ib, o_ps)
                while proj_pending:
                    emit_proj_unit(*proj_pending.pop(0))

    nc.compile()
    return nc


# ------------------------------------------------------------------- driver

def _rope_tables():
    half = DH // 2
    inv_freq = 1.0 / (ROPE_BASE ** (np.arange(half, dtype=np.float64) * 2.0
                                    / DH))
    freqs = np.arange(N, dtype=np.float64)[:, None] * inv_freq[None, :]
    cos = np.cos(freqs).T          # (32, N)
    sin = np.sin(freqs).T
    cos64 = np.concatenate([cos, cos], 0)            # (64, N)
    sin64 = np.concatenate([-sin, sin], 0)           # signed for rotate_half
    cos_t = np.ascontiguousarray(
        np.concatenate([cos64, cos64], 0).astype(BF16NP))  # (128, N)
    sin_t = np.ascontiguousarray(
        np.concatenate([sin64, sin64], 0).astype(BF16NP))
    return cos_t, sin_t


def kernel(input, w_qkv, b_qkv, q_scale, k_scale, w_out, b_out):
    trace = bool(os.environ.get("KERNEL_TRACE"))
    if "fused" not in _cache:
        _cache["fused"] = _build_fused()

    x = np.asarray(input, dtype=np.float32)
    w_qkv = np.asarray(w_qkv, dtype=np.float32)
    b_qkv = np.asarray(b_qkv, dtype=np.float32)
    qs = np.asarray(q_scale, dtype=np.float32)
    ks = np.asarray(k_scale, dtype=np.float32)
    w_out = np.asarray(w_out, dtype=np.float32)
    b_out = np.asarray(b_out, dtype=np.float32)

    wq = w_qkv[:, :DQ] * qs[None, :]
    wk = w_qkv[:, DQ:2 * DQ] * ks[None, :]
    wv = w_qkv[:, 2 * DQ:]
    bq = b_qkv[:DQ] * qs
    bk = b_qkv[DQ:2 * DQ] * ks
    bv = b_qkv[2 * DQ:]

    cos_t, sin_t = _rope_tables()
    xT = [np.ascontiguousarray(x[b].T.astype(BF16NP)) for b in range(B)]

    def col4(vec256_a, vec256_b):
        # -> (128, 4): [a_mt0 | a_mt1 | b_mt0 | b_mt1]
        return np.ascontiguousarray(np.stack(
            [vec256_a[:P], vec256_a[P:], vec256_b[:P], vec256_b[P:]],
            axis=1).astype(np.float32))

    ins = []
    for c in range(NCORES):
        b, g = divmod(c, NGROUP)
        sl = slice(g * GF, (g + 1) * GF)
        wcat = np.ascontiguousarray(np.concatenate(
            [wq[:, sl], wk[:, sl], wv[:, sl]], axis=1).astype(BF16NP))
        ins.append({
            "xT": xT[b],
            "wcat": wcat,
            "bqk": col4(bq[sl], bk[sl]),
            "winv": np.ascontiguousarray(np.repeat(
                col4(1.0 / np.square(qs[sl]), 1.0 / np.square(ks[sl])),
                32, axis=1).astype(BF16NP)),
            "cos_t": cos_t,
            "sin_t": sin_t,
            "wout": np.ascontiguousarray(w_out[sl, :].astype(BF16NP)),
        })

    r = run_bass_kernel_spmd(_cache["fused"], ins,
                             core_ids=list(range(NCORES)), trace=trace)
    if trace:
        LAST_EXEC_NS["fused"] = r.exec_time_ns
        LAST_RESULTS["fused"] = r

    base = (bv.astype(np.float64) @ w_out.astype(np.float64)
            + b_out.astype(np.float64))
    out = np.zeros((B, N, DIN), np.float32)
    for b in range(B):
        acc = np.zeros((N, DIN), np.float64)
        for g in range(NGROUP):
            p = r.results[NGROUP * b + g]["part"].astype(np.float64)
            acc += p[0]
            acc += p[1]
        out[b] = (acc + base[None, :]).astype(np.float32)
    return out


# revision 19
# speedup vs baseline: 1.2155x; 1.1076x over previous
"""DiT attention block on 8 Trainium2 NeuronCores — fused single launch.

Reference computation (fp32):
    qkv = x @ Wqkv + b            (b=2, n=2048, din=1024, 3*1024)
    q, k = RMSNorm_full_dim(q|k) * scale  (norm over all 1024 channels)
    RoPE (rotary_dim=64) per 64-dim head, 16 heads
    attn = softmax(q k^T / 8) v ;  out = attn @ Wout + bout

Sharding: 8 cores = 2 batches x 4 head-groups (4 heads / 256 features each).
ONE SPMD launch per core:
  k-proj -> ssq_k partial -> AllReduce([0-3],[4-7]) ; q-proj -> ssq_q ->
  AllReduce ; v-proj (PE stays dense) while rsqrt + rope run on ACT/DVE/
  GpSimd; then the flattened attention stream (S matmul -> exp on ACT ->
  AV matmul with softmax denominator as a 65th V column), out-projection
  partials injected into the stream as each 512-query block normalizes.
Host: preps transposed/bf16 inputs, sums the 8x2 projection partials and
adds the (host-folded) v-bias/out-bias term.

Precision: matmul inputs bf16 except qhat/khat (fp32r) so the softmax
logits stay accurate; PSUM accumulation fp32 everywhere; the RMSNorm
rsqrt and softmax reciprocal run at fp32 (reciprocal_approx_fast).
"""

import os
import sys

for _p in ("/opt/trn_rl_repo", "/root/.axon_site/_ro/trn_rl_repo"):
    if os.path.isdir(_p) and _p not in sys.path:
        sys.path.append(_p)

import numpy as np
import ml_dtypes

import concourse.bass as bass  # noqa: E402,F401
import concourse.mybir as mybir  # noqa: E402
import concourse.tile as tile  # noqa: E402
from concourse import bacc  # noqa: E402
from concourse.bass_utils import run_bass_kernel_spmd  # noqa: E402

FP32 = mybir.dt.float32
FP32R = mybir.dt.float32r
BF16 = mybir.dt.bfloat16
AF = mybir.ActivationFunctionType
BF16NP = ml_dtypes.bfloat16

B = 2
N = 2048
DIN = 1024
DQ = 1024
H = 16
DH = 64
NCORES = 8
NGROUP = 4          # head-groups per batch
GF = 256            # features per core (4 heads)
P = 128
EPS = 1e-6
ROPE_BASE = 10000.0

IBW = 512           # query-block width
NIB = N // IBW      # 4 query blocks
NJT = N // P        # 16 key tiles
KT = DIN // P       # 8 contraction tiles

LAST_EXEC_NS = {}   # filled when KERNEL_TRACE=1
LAST_RESULTS = {}

_cache = {}


def _build_fused():
    nc = bacc.Bacc("TRN2", target_bir_lowering=False, debug=False,
                   num_devices=NCORES)
    xT_i = nc.dram_tensor("xT", [DIN, N], BF16, kind="ExternalInput")
    wcat_i = nc.dram_tensor("wcat", [DIN, 3 * GF], BF16, kind="ExternalInput")
    bqk_i = nc.dram_tensor("bqk", [P, 4], FP32, kind="ExternalInput")
    winv_i = nc.dram_tensor("winv", [P, P], BF16, kind="ExternalInput")
    cos_i = nc.dram_tensor("cos_t", [P, N], BF16, kind="ExternalInput")
    sin_i = nc.dram_tensor("sin_t", [P, N], BF16, kind="ExternalInput")
    wout_i = nc.dram_tensor("wout", [GF, DIN], BF16, kind="ExternalInput")
    part_o = nc.dram_tensor("part", [2, N, DIN], BF16, kind="ExternalOutput")

    groups = [[0, 1, 2, 3], [4, 5, 6, 7]]

    with tile.TileContext(nc) as tc:
        with (
            tc.tile_pool(name="cst", bufs=1) as cst,
            tc.tile_pool(name="hat", bufs=1) as hatp,
            tc.tile_pool(name="obig", bufs=1) as obigp,
            tc.tile_pool(name="dram", bufs=1, space="DRAM") as dram,
        ):
            # ---------------- constant loads ----------------
            xt, wt = [], []
            for kt in range(KT):
                t = cst.tile([P, N], BF16, tag=f"xt{kt}", name=f"xt{kt}")
                nc.sync.dma_start(t[:], xT_i[kt * P:(kt + 1) * P, :])
                xt.append(t)
                w = cst.tile([P, 3 * GF], BF16, tag=f"wt{kt}", name=f"wt{kt}")
                nc.sync.dma_start(w[:], wcat_i[kt * P:(kt + 1) * P, :])
                wt.append(w)
            bias = cst.tile([P, 4], FP32, tag="bias")
            nc.sync.dma_start(bias[:], bqk_i[:, :])
            winv = cst.tile([P, P], BF16, tag="winv")
            nc.sync.dma_start(winv[:], winv_i[:, :])
            cos_sb = cst.tile([P, N], BF16, tag="cos_sb")
            nc.sync.dma_start(cos_sb[:], cos_i[:, :])
            sin_sb = cst.tile([P, N], BF16, tag="sin_sb")
            nc.sync.dma_start(sin_sb[:], sin_i[:, :])
            wout = []
            for kt in range(2):
                w = cst.tile([P, DIN], BF16, tag=f"wo{kt}", name=f"wo{kt}")
                nc.sync.dma_start(w[:], wout_i[kt * P:(kt + 1) * P, :])
                wout.append(w)
            # packed V tiles: per vb, 4 token-subtiles x 4 heads x 65 cols
            # (64 v-features + a ones column for the softmax denominator)
            vbig = [cst.tile([P, 4 * 4 * 65], BF16, tag=f"v{vb}",
                             name=f"v{vb}")
                    for vb in range(NJT // 4)]
            for vb in range(NJT // 4):
                ones_ap = vbig[vb][:].rearrange(
                    "p (g c) -> p g c", g=16)[:, :, 64:65]
                nc.vector.memset(ones_ap, 1.0)

            def vt_slice(jt, h):
                base = (jt % 4) * 4 * 65 + h * 65
                return vbig[jt // 4][:, base:base + 65]

            # CC bounce buffers (DRAM)
            ssq_in = [dram.tile([1, N], FP32, tag=f"cci{t}", name=f"cci{t}")
                      for t in range(2)]
            ssq_out = [dram.tile([1, N], FP32, tag=f"cco{t}", name=f"cco{t}")
                       for t in range(2)]
            # warm-up collective: absorbs the ~20us first-CC mesh setup and
            # aligns the cores before the real reductions
            ccw_in = dram.tile([1, 64], FP32, tag="ccwi", name="ccwi")
            ccw_out = dram.tile([1, 64], FP32, tag="ccwo", name="ccwo")
            ccw_sb = cst.tile([1, 64], FP32, tag="ccwsb")
            nc.vector.memset(ccw_sb[:], 0.0)
            nc.gpsimd.dma_start(ccw_in[:], ccw_sb[:])
            nc.gpsimd.collective_compute(
                "AllReduce", mybir.AluOpType.add, replica_groups=groups,
                ins=[ccw_in[:]], outs=[ccw_out[:]])

            obig = [obigp.tile([P, N], BF16, tag=f"obig{pr}", name=f"ob{pr}")
                    for pr in range(2)]

            # ---------------- phase 1: qkv projections + ssq ----------------
            prime = {}
            hats = {}
            with (
                tc.tile_pool(name="prm", bufs=1) as prm,
                tc.tile_pool(name="rp", bufs=1) as rp,
                tc.tile_pool(name="scr", bufs=1) as scr,
                tc.tile_pool(name="sqp", bufs=2) as sqp,
                tc.tile_pool(name="stgp", bufs=2) as stgp,
                tc.tile_pool(name="ps", bufs=2, space="PSUM") as ps,
                tc.tile_pool(name="pssq", bufs=2, space="PSUM") as pssq,
                tc.tile_pool(name="psv", bufs=2, space="PSUM") as psv,
            ):
                def emit_ssq(name, t_idx, nb):
                    # weighted sum-of-squares partial, DMA'd straight from
                    # PSUM into the collective's DRAM bounce buffer
                    sp = pssq.tile([32, 512], FP32, tag="ssq",
                                   name=f"ssq{t_idx}_{nb}")
                    for mt in range(2):
                        sq = sqp.tile([P, 512], BF16, tag="sq")
                        nc.vector.tensor_mul(
                            sq[:],
                            prime[(name, mt)][:, nb * 512:(nb + 1) * 512],
                            prime[(name, mt)][:, nb * 512:(nb + 1) * 512])
                        nc.tensor.matmul(
                            sp[:],
                            winv[:, 32 * (2 * t_idx + mt):
                                 32 * (2 * t_idx + mt + 1)],
                            sq[:],
                            start=(mt == 0),
                            stop=(mt == 1),
                        )
                    stg = stgp.tile([1, 512], FP32, tag="stg",
                                    name=f"stg{t_idx}_{nb}")
                    nc.vector.tensor_copy(stg[:], sp[0:1, :])
                    nc.gpsimd.dma_start(
                        ssq_in[t_idx][0:1, nb * 512:(nb + 1) * 512],
                        stg[:])

                def emit_rope(name, mt):
                    # UNSCALED rope (r folded in after the collective): runs
                    # with no dependency on the AllReduce
                    big = prime[(name, mt)]
                    sh = scr.tile([P, N], BF16, tag="sh", bufs=2,
                                  name=f"sh{name}{mt}")
                    for blk in range(4):
                        srcb = blk ^ 1
                        nc.sync.dma_start(
                            sh[blk * 32:(blk + 1) * 32, :],
                            big[srcb * 32:(srcb + 1) * 32, :])
                    # in place: prime's other readers (ssq squares and the
                    # shift DMAs above) are already issued
                    nc.vector.tensor_mul(big[:], big[:], cos_sb[:])
                    nc.gpsimd.tensor_mul(sh[:], sh[:], sin_sb[:])
                    hat = hatp.tile([P, N], FP32R, tag=f"hat_{name}{mt}",
                                    name=f"hat_{name}{mt}")
                    nc.vector.tensor_add(hat[:], big[:], sh[:])
                    hats[(name, mt)] = hat

                # k first so its AllReduce is in flight the longest
                for name, t_idx in (("k", 1), ("q", 0)):
                    col0 = GF if t_idx == 1 else 0
                    for mt in range(2):
                        prime[(name, mt)] = prm.tile(
                            [P, N], BF16, tag=f"{name}{mt}",
                            name=f"{name}{mt}")
                    for nb in range(4):
                        for mt in range(2):
                            big = prime[(name, mt)]
                            acc = ps.tile([P, 512], FP32, tag="acc")
                            for kt in range(KT):
                                nc.tensor.matmul(
                                    acc[:],
                                    wt[kt][:, col0 + mt * P:
                                           col0 + (mt + 1) * P],
                                    xt[kt][:, nb * 512:(nb + 1) * 512],
                                    start=(kt == 0),
                                    stop=(kt == KT - 1),
                                )
                            nc.scalar.activation(
                                big[:, nb * 512:(nb + 1) * 512], acc[:],
                                AF.Identity,
                                bias=bias[:, 2 * t_idx + mt:
                                          2 * t_idx + mt + 1],
                            )
                        if nb >= 1:
                            emit_ssq(name, t_idx, nb - 1)
                    emit_ssq(name, t_idx, 3)
                    nc.gpsimd.collective_compute(
                        "AllReduce",
                        mybir.AluOpType.add,
                        replica_groups=groups,
                        ins=[ssq_in[t_idx][:]],
                        outs=[ssq_out[t_idx][:]],
                    )
                    emit_rope(name, 0)
                    emit_rope(name, 1)

                # ---- v projection (fills the collective-wait window; the
                # psum->vbig drains alternate ACT/DVE so neither engine's
                # in-order queue gates the v matmul cadence) ----
                for tt in range(NJT):
                    acc = psv.tile([P, GF], FP32, tag="vacc")
                    for kt in range(KT):
                        nc.tensor.matmul(
                            acc[:],
                            xt[kt][:, tt * P:(tt + 1) * P],
                            wt[kt][:, 2 * GF:3 * GF],
                            start=(kt == 0),
                            stop=(kt == KT - 1),
                        )
                    out_ap = vbig[tt // 4][:].rearrange(
                        "p (g c) -> p g c",
                        g=16)[:, 4 * (tt % 4):4 * (tt % 4) + 4, 0:64]
                    in_ap = acc[:].rearrange("p (h d) -> p h d", h=4)
                    if tt % 2 == 0:
                        nc.scalar.activation(out_ap, in_ap, AF.Identity)
                    else:
                        nc.vector.tensor_copy(out_ap, in_ap)

                # post-collective: r = rsqrt(mean+eps), broadcast, fold into
                # the finished (unscaled) hats with one in-place mul each
                for name, t_idx in (("k", 1), ("q", 0)):
                    st = rp.tile([1, N], FP32, tag="st", bufs=2,
                                 name=f"st{name}")
                    nc.gpsimd.dma_start(st[:], ssq_out[t_idx][:])
                    nc.vector.tensor_scalar(
                        st[:], st[:], 1.0 / DQ, EPS,
                        mybir.AluOpType.mult, mybir.AluOpType.add)
                    nc.scalar.activation(st[:], st[:], AF.Sqrt)
                    nc.vector.reciprocal_approx_fast(out=st[:], in_=st[:])
                    bcr = rp.tile([P, N], FP32, tag="bcr", bufs=2,
                                  name=f"bcr{name}")
                    nc.gpsimd.partition_broadcast(bcr[:, :], st[:, :])
                    for mt in range(2):
                        hat = hats[(name, mt)]
                        nc.vector.tensor_mul(hat[:], hat[:], bcr[:])

            # ---------------- phase 2: attention ----------------
            with (
                tc.tile_pool(name="ptp", bufs=4) as ptp,
                tc.tile_pool(name="onrm", bufs=2) as onrm,
                tc.tile_pool(name="outp", bufs=2) as outp,
                tc.tile_pool(name="tiny", bufs=2) as tiny,
                tc.tile_pool(name="psS", bufs=2, space="PSUM") as psS,
                tc.tile_pool(name="psO", bufs=4, space="PSUM") as psO,
            ):
                osb_pend = {}
                proj_pending = []

                def emit_proj_unit(pr, tt, half):
                    pss = psO.tile([P, 512], FP32, tag="O",
                                   name=f"pj{pr}_{tt}_{half}")
                    nc.tensor.matmul(
                        pss[:],
                        obig[pr][:, tt * P:(tt + 1) * P],
                        wout[pr][:, half * 512:(half + 1) * 512],
                        start=True, stop=True,
                    )
                    if (tt % 2, half) == (0, 0):
                        osb_pend[pr] = outp.tile([P, 2 * DIN], BF16,
                                                 tag="osb",
                                                 name=f"osb{pr}_{tt}")
                    osb = osb_pend[pr]
                    base = (tt % 2) * DIN + half * 512
                    nc.vector.tensor_copy(osb[:, base:base + 512], pss[:])
                    if (tt % 2, half) == (1, 1):
                        # gpsimd queue: keeps the bulky output DMAs from
                        # head-of-line-blocking the sync queue's small hops
                        nc.gpsimd.dma_start(
                            part_o[pr, (tt - 1) * P:(tt + 1) * P, :]
                            .rearrange("(a p) d -> p a d", p=P),
                            osb[:].rearrange("p (a d) -> p a d", a=2))

                def finish_block(pr, ib, o_ps):
                    # row 64 of each o_ps is the softmax denominator.
                    rrd, rr0, bc = [], [], []
                    for sub in range(2):
                        t = tiny.tile([65, IBW], FP32, tag="rrd",
                                      name=f"rrd{pr}_{ib}_{sub}")
                        nc.vector.tensor_copy(t[64:65, :],
                                              o_ps[sub][64:65, :])
                        rrd.append(t)
                    for sub in range(2):
                        t = tiny.tile([1, IBW], FP32, tag="rr0",
                                      name=f"rr0{pr}_{ib}_{sub}")
                        nc.sync.dma_start(t[:, :], rrd[sub][64:65, :])
                        rr0.append(t)
                    for sub in range(2):
                        nc.vector.reciprocal_approx_fast(
                            out=rr0[sub][:, :], in_=rr0[sub][:, :])
                    for sub in range(2):
                        t = tiny.tile([64, IBW], FP32, tag="bc",
                                      name=f"bc{pr}_{ib}_{sub}")
                        nc.gpsimd.partition_broadcast(t[:, :], rr0[sub][:, :])
                        bc.append(t)
                    nc.vector.tensor_mul(
                        obig[pr][0:64, ib * IBW:(ib + 1) * IBW],
                        o_ps[0][0:64, :], bc[0][:, :])
                    onr = onrm.tile([64, IBW], BF16, tag="onr")
                    nc.vector.tensor_mul(onr[:, :], o_ps[1][0:64, :],
                                         bc[1][:, :])
                    nc.sync.dma_start(
                        obig[pr][64:128, ib * IBW:(ib + 1) * IBW], onr[:, :])
                    # queue this block's out-projection into the stream
                    for tt in range(ib * 4, ib * 4 + 4):
                        for half in range(2):
                            proj_pending.append((pr, tt, half))

                steps = [(pr, ib, jt) for pr in range(2) for ib in range(NIB)
                         for jt in range(NJT)]
                p_sbs = {}
                o_ps_map = {}

                def emit_s(step):
                    pr, ib, jt = step
                    s_ps = psS.tile([P, 2 * IBW], FP32, tag="S")
                    for sub in range(2):
                        nc.tensor.matmul(
                            s_ps[:, sub * IBW:(sub + 1) * IBW],
                            hats[("k", pr)][sub * 64:(sub + 1) * 64,
                                            jt * P:(jt + 1) * P],
                            hats[("q", pr)][sub * 64:(sub + 1) * 64,
                                            ib * IBW:(ib + 1) * IBW],
                            start=True, stop=True,
                            tile_position=(64 * sub, 0),
                        )
                    p_sb = ptp.tile([P, 2 * IBW], BF16, tag="P",
                                    name=f"p{pr}_{ib}_{jt}")
                    nc.scalar.activation(p_sb[:, :], s_ps[:, :],
                                         AF.Exp, scale=0.125)
                    p_sbs[step] = p_sb

                def emit_av(step):
                    # AV lags one step behind S/exp so the in-order PE never
                    # waits on the current step's exp
                    pr, ib, jt = step
                    if (pr, ib) not in o_ps_map:
                        o_ps_map[(pr, ib)] = [
                            psO.tile([65, IBW], FP32, tag="O",
                                     name=f"o{pr}_{ib}_{s}")
                            for s in range(2)]
                    o_ps = o_ps_map[(pr, ib)]
                    p_sb = p_sbs.pop(step)
                    for sub in range(2):
                        h = 2 * pr + sub
                        nc.tensor.matmul(
                            o_ps[sub][:, :],
                            vt_slice(jt, h),
                            p_sb[:, sub * IBW:(sub + 1) * IBW],
                            start=(jt == 0), stop=(jt == NJT - 1),
                        )
                    if jt == NJT - 1:
                        finish_block(pr, ib, o_ps)

                emit_s(steps[0])
                for si, step in enumerate(steps):
                    if si + 1 < len(steps):
                        emit_s(steps[si + 1])
                    if si >= 1:
                        emit_av(steps[si - 1])
                    if proj_pending and si % 2 == 1:
                        emit_proj_unit(*proj_pending.pop(0))
                emit_av(steps[-1])
                while proj_pending:
                    emit_proj_unit(*proj_pending.pop(0))

    nc.compile()
    return nc


# ------------------------------------------------------------------- driver

def _rope_tables():
    half = DH // 2
    inv_freq = 1.0 / (ROPE_BASE ** (np.arange(half, dtype=np.float64) * 2.0
                                    / DH))
    freqs = np.arange(N, dtype=np.float64)[:, None] * inv_freq[None, :]
    cos = np.cos(freqs).T          # (32, N)
    sin = np.sin(freqs).T
    cos64 = np.concatenate([cos, cos], 0)            # (64, N)
    sin64 = np.concatenate([-sin, sin], 0)           # signed for rotate_half
    cos_t = np.ascontiguousarray(
        np.concatenate([cos64, cos64], 0).astype(BF16NP))  # (128, N)
    sin_t = np.ascontiguousarray(
        np.concatenate([sin64, sin64], 0).astype(BF16NP))
    return cos_t, sin_t


def kernel(input, w_qkv, b_qkv, q_scale, k_scale, w_out, b_out):
    trace = bool(os.environ.get("KERNEL_TRACE"))
    if "fused" not in _cache:
        _cache["fused"] = _build_fused()

    x = np.asarray(input, dtype=np.float32)
    w_qkv = np.asarray(w_qkv, dtype=np.float32)
    b_qkv = np.asarray(b_qkv, dtype=np.float32)
    qs = np.asarray(q_scale, dtype=np.float32)
    ks = np.asarray(k_scale, dtype=np.float32)
    w_out = np.asarray(w_out, dtype=np.float32)
    b_out = np.asarray(b_out, dtype=np.float32)

    wq = w_qkv[:, :DQ] * qs[None, :]
    wk = w_qkv[:, DQ:2 * DQ] * ks[None, :]
    wv = w_qkv[:, 2 * DQ:]
    bq = b_qkv[:DQ] * qs
    bk = b_qkv[DQ:2 * DQ] * ks
    bv = b_qkv[2 * DQ:]

    cos_t, sin_t = _rope_tables()
    xT = [np.ascontiguousarray(x[b].T.astype(BF16NP)) for b in range(B)]

    def col4(vec256_a, vec256_b):
        # -> (128, 4): [a_mt0 | a_mt1 | b_mt0 | b_mt1]
        return np.ascontiguousarray(np.stack(
            [vec256_a[:P], vec256_a[P:], vec256_b[:P], vec256_b[P:]],
            axis=1).astype(np.float32))

    ins = []
    for c in range(NCORES):
        b, g = divmod(c, NGROUP)
        sl = slice(g * GF, (g + 1) * GF)
        wcat = np.ascontiguousarray(np.concatenate(
            [wq[:, sl], wk[:, sl], wv[:, sl]], axis=1).astype(BF16NP))
        ins.append({
            "xT": xT[b],
            "wcat": wcat,
            "bqk": col4(bq[sl], bk[sl]),
            "winv": np.ascontiguousarray(np.repeat(
                col4(1.0 / np.square(qs[sl]), 1.0 / np.square(ks[sl])),
                32, axis=1).astype(BF16NP)),
            "cos_t": cos_t,
            "sin_t": sin_t,
            "wout": np.ascontiguousarray(w_out[sl, :].astype(BF16NP)),
        })

    r = run_bass_kernel_spmd(_cache["fused"], ins,
                             core_ids=list(range(NCORES)), trace=trace)
    if trace:
        LAST_EXEC_NS["fused"] = r.exec_time_ns
        LAST_RESULTS["fused"] = r

    base = (bv.astype(np.float64) @ w_out.astype(np.float64)
            + b_out.astype(np.float64))
    out = np.zeros((B, N, DIN), np.float32)
    for b in range(B):
        acc = np.zeros((N, DIN), np.float64)
        for g in range(NGROUP):
            p = r.results[NGROUP * b + g]["part"].astype(np.float64)
            acc += p[0]
            acc += p[1]
        out[b] = (acc + base[None, :]).astype(np.float32)
    return out


# revision 22
# speedup vs baseline: 1.2261x; 1.0088x over previous
"""DiT attention block on 8 Trainium2 NeuronCores — fused single launch.

Reference computation (fp32):
    qkv = x @ Wqkv + b            (b=2, n=2048, din=1024, 3*1024)
    q, k = RMSNorm_full_dim(q|k) * scale  (norm over all 1024 channels)
    RoPE (rotary_dim=64) per 64-dim head, 16 heads
    attn = softmax(q k^T / 8) v ;  out = attn @ Wout + bout

Sharding: 8 cores = 2 batches x 4 head-groups (4 heads / 256 features each).
ONE SPMD launch per core:
  k-proj -> ssq_k partial -> AllReduce([0-3],[4-7]) ; q-proj -> ssq_q ->
  AllReduce ; v-proj (PE stays dense) while rsqrt + rope run on ACT/DVE/
  GpSimd; then the flattened attention stream (S matmul -> exp on ACT ->
  AV matmul with softmax denominator as a 65th V column), out-projection
  partials injected into the stream as each 512-query block normalizes.
Host: preps transposed/bf16 inputs, sums the 8x2 projection partials and
adds the (host-folded) v-bias/out-bias term.

Precision: matmul inputs bf16 except qhat/khat (fp32r) so the softmax
logits stay accurate; PSUM accumulation fp32 everywhere; the RMSNorm
rsqrt and softmax reciprocal run at fp32 (reciprocal_approx_fast).
"""

import os
import sys

for _p in ("/opt/trn_rl_repo", "/root/.axon_site/_ro/trn_rl_repo"):
    if os.path.isdir(_p) and _p not in sys.path:
        sys.path.append(_p)

import numpy as np
import ml_dtypes

import concourse.bass as bass  # noqa: E402,F401
import concourse.mybir as mybir  # noqa: E402
import concourse.tile as tile  # noqa: E402
from concourse import bacc  # noqa: E402
from concourse.bass_utils import run_bass_kernel_spmd  # noqa: E402

FP32 = mybir.dt.float32
FP32R = mybir.dt.float32r
BF16 = mybir.dt.bfloat16
AF = mybir.ActivationFunctionType
BF16NP = ml_dtypes.bfloat16

B = 2
N = 2048
DIN = 1024
DQ = 1024
H = 16
DH = 64
NCORES = 8
NGROUP = 4          # head-groups per batch
GF = 256            # features per core (4 heads)
P = 128
EPS = 1e-6
ROPE_BASE = 10000.0

IBW = 512           # query-block width
NIB = N // IBW      # 4 query blocks
NJT = N // P        # 16 key tiles
KT = DIN // P       # 8 contraction tiles

LAST_EXEC_NS = {}   # filled when KERNEL_TRACE=1
LAST_RESULTS = {}

_cache = {}


def _build_fused():
    nc = bacc.Bacc("TRN2", target_bir_lowering=False, debug=False,
                   num_devices=NCORES)
    xT_i = nc.dram_tensor("xT", [DIN, N], BF16, kind="ExternalInput")
    wcat_i = nc.dram_tensor("wcat", [DIN, 3 * GF], BF16, kind="ExternalInput")
    bqk_i = nc.dram_tensor("bqk", [P, 4], FP32, kind="ExternalInput")
    winv_i = nc.dram_tensor("winv", [P, P], BF16, kind="ExternalInput")
    cos_i = nc.dram_tensor("cos_t", [P, N], BF16, kind="ExternalInput")
    sin_i = nc.dram_tensor("sin_t", [P, N], BF16, kind="ExternalInput")
    wout_i = nc.dram_tensor("wout", [GF, DIN], BF16, kind="ExternalInput")
    part_o = nc.dram_tensor("part", [2, N, DIN], BF16, kind="ExternalOutput")

    groups = [[0, 1, 2, 3], [4, 5, 6, 7]]

    with tile.TileContext(nc) as tc:
        with (
            tc.tile_pool(name="cst", bufs=1) as cst,
            tc.tile_pool(name="hat", bufs=1) as hatp,
            tc.tile_pool(name="obig", bufs=1) as obigp,
            tc.tile_pool(name="dram", bufs=1, space="DRAM") as dram,
        ):
            # ---------------- constant loads ----------------
            xt, wt = [], []
            for kt in range(KT):
                t = cst.tile([P, N], BF16, tag=f"xt{kt}", name=f"xt{kt}")
                nc.sync.dma_start(t[:], xT_i[kt * P:(kt + 1) * P, :])
                xt.append(t)
                w = cst.tile([P, 3 * GF], BF16, tag=f"wt{kt}", name=f"wt{kt}")
                nc.gpsimd.dma_start(w[:], wcat_i[kt * P:(kt + 1) * P, :])
                wt.append(w)
            bias = cst.tile([P, 4], FP32, tag="bias")
            nc.sync.dma_start(bias[:], bqk_i[:, :])
            winv = cst.tile([P, P], BF16, tag="winv")
            nc.sync.dma_start(winv[:], winv_i[:, :])
            cos_sb = cst.tile([P, N], BF16, tag="cos_sb")
            nc.sync.dma_start(cos_sb[:], cos_i[:, :])
            sin_sb = cst.tile([P, N], BF16, tag="sin_sb")
            nc.sync.dma_start(sin_sb[:], sin_i[:, :])
            wout = []
            for kt in range(2):
                w = cst.tile([P, DIN], BF16, tag=f"wo{kt}", name=f"wo{kt}")
                nc.sync.dma_start(w[:], wout_i[kt * P:(kt + 1) * P, :])
                wout.append(w)
            # packed V tiles: per vb, 4 token-subtiles x 4 heads x 65 cols
            # (64 v-features + a ones column for the softmax denominator)
            vbig = [cst.tile([P, 4 * 4 * 65], BF16, tag=f"v{vb}",
                             name=f"v{vb}")
                    for vb in range(NJT // 4)]
            for vb in range(NJT // 4):
                ones_ap = vbig[vb][:].rearrange(
                    "p (g c) -> p g c", g=16)[:, :, 64:65]
                nc.vector.memset(ones_ap, 1.0)

            def vt_slice(jt, h):
                base = (jt % 4) * 4 * 65 + h * 65
                return vbig[jt // 4][:, base:base + 65]

            # CC bounce buffers (DRAM)
            ssq_in = [dram.tile([1, N], FP32, tag=f"cci{t}", name=f"cci{t}")
                      for t in range(2)]
            ssq_out = [dram.tile([1, N], FP32, tag=f"cco{t}", name=f"cco{t}")
                       for t in range(2)]
            # warm-up collective: absorbs the ~20us first-CC mesh setup and
            # aligns the cores before the real reductions
            ccw_in = dram.tile([1, 64], FP32, tag="ccwi", name="ccwi")
            ccw_out = dram.tile([1, 64], FP32, tag="ccwo", name="ccwo")
            ccw_sb = cst.tile([1, 64], FP32, tag="ccwsb")
            nc.vector.memset(ccw_sb[:], 0.0)
            # pre-warm the ACT tables used later so no mid-kernel
            # ACT_TABLE_LOAD lands on the critical path
            warm = cst.tile([1, 64], FP32, tag="warm")
            nc.scalar.activation(warm[:], ccw_sb[:], AF.Identity)
            nc.scalar.activation(warm[:], ccw_sb[:], AF.Sqrt)
            nc.scalar.activation(warm[:], ccw_sb[:], AF.Exp)
            nc.gpsimd.dma_start(ccw_in[:], ccw_sb[:])
            nc.gpsimd.collective_compute(
                "AllReduce", mybir.AluOpType.add, replica_groups=groups,
                ins=[ccw_in[:]], outs=[ccw_out[:]])

            obig = [obigp.tile([P, N], BF16, tag=f"obig{pr}", name=f"ob{pr}")
                    for pr in range(2)]

            # ---------------- phase 1: qkv projections + ssq ----------------
            prime = {}
            hats = {}
            with (
                tc.tile_pool(name="prm", bufs=1) as prm,
                tc.tile_pool(name="rp", bufs=1) as rp,
                tc.tile_pool(name="scr", bufs=1) as scr,
                tc.tile_pool(name="sqp", bufs=2) as sqp,
                tc.tile_pool(name="stgp", bufs=2) as stgp,
                tc.tile_pool(name="ps", bufs=3, space="PSUM") as ps,
                tc.tile_pool(name="pssq", bufs=2, space="PSUM") as pssq,
                tc.tile_pool(name="psv", bufs=2, space="PSUM") as psv,
            ):
                def emit_ssq(name, t_idx, nb):
                    # weighted sum-of-squares partial, DMA'd straight from
                    # PSUM into the collective's DRAM bounce buffer
                    sp = pssq.tile([32, 512], FP32, tag="ssq",
                                   name=f"ssq{t_idx}_{nb}")
                    for mt in range(2):
                        sq = sqp.tile([P, 512], BF16, tag="sq")
                        nc.vector.tensor_mul(
                            sq[:],
                            prime[(name, mt)][:, nb * 512:(nb + 1) * 512],
                            prime[(name, mt)][:, nb * 512:(nb + 1) * 512])
                        nc.tensor.matmul(
                            sp[:],
                            winv[:, 32 * (2 * t_idx + mt):
                                 32 * (2 * t_idx + mt + 1)],
                            sq[:],
                            start=(mt == 0),
                            stop=(mt == 1),
                        )
                    stg = stgp.tile([1, 512], FP32, tag="stg",
                                    name=f"stg{t_idx}_{nb}")
                    nc.scalar.activation(stg[:], sp[0:1, :], AF.Identity)
                    nc.gpsimd.dma_start(
                        ssq_in[t_idx][0:1, nb * 512:(nb + 1) * 512],
                        stg[:])

                def emit_rope(name, mt):
                    # UNSCALED rope (r folded in after the collective): runs
                    # with no dependency on the AllReduce
                    big = prime[(name, mt)]
                    sh = scr.tile([P, N], BF16, tag="sh", bufs=2,
                                  name=f"sh{name}{mt}")
                    for blk in range(4):
                        srcb = blk ^ 1
                        nc.sync.dma_start(
                            sh[blk * 32:(blk + 1) * 32, :],
                            big[srcb * 32:(srcb + 1) * 32, :])
                    # in place: prime's other readers (ssq squares and the
                    # shift DMAs above) are already issued
                    nc.vector.tensor_mul(big[:], big[:], cos_sb[:])
                    nc.gpsimd.tensor_mul(sh[:], sh[:], sin_sb[:])
                    hat = hatp.tile([P, N], FP32R, tag=f"hat_{name}{mt}",
                                    name=f"hat_{name}{mt}")
                    nc.vector.tensor_add(hat[:], big[:], sh[:])
                    hats[(name, mt)] = hat

                # k first so its AllReduce is in flight the longest
                for name, t_idx in (("k", 1), ("q", 0)):
                    col0 = GF if t_idx == 1 else 0
                    for mt in range(2):
                        prime[(name, mt)] = prm.tile(
                            [P, N], BF16, tag=f"{name}{mt}",
                            name=f"{name}{mt}")
                    for nb in range(4):
                        for mt in range(2):
                            big = prime[(name, mt)]
                            acc = ps.tile([P, 512], FP32, tag="acc")
                            for kt in range(KT):
                                nc.tensor.matmul(
                                    acc[:],
                                    wt[kt][:, col0 + mt * P:
                                           col0 + (mt + 1) * P],
                                    xt[kt][:, nb * 512:(nb + 1) * 512],
                                    start=(kt == 0),
                                    stop=(kt == KT - 1),
                                )
                            nc.scalar.activation(
                                big[:, nb * 512:(nb + 1) * 512], acc[:],
                                AF.Identity,
                                bias=bias[:, 2 * t_idx + mt:
                                          2 * t_idx + mt + 1],
                            )
                        if nb >= 1:
                            emit_ssq(name, t_idx, nb - 1)
                    emit_ssq(name, t_idx, 3)
                    nc.gpsimd.collective_compute(
                        "AllReduce",
                        mybir.AluOpType.add,
                        replica_groups=groups,
                        ins=[ssq_in[t_idx][:]],
                        outs=[ssq_out[t_idx][:]],
                    )

                # ropes only issue after BOTH collectives are fed, so no
                # engine queue has CC-dependent work ahead of CC-feeding work
                def emit_rsqrt(name, t_idx):
                    st = rp.tile([1, N], FP32, tag="st", bufs=2,
                                 name=f"st{name}")
                    nc.gpsimd.dma_start(st[:], ssq_out[t_idx][:])
                    nc.vector.tensor_scalar(
                        st[:], st[:], 1.0 / DQ, EPS,
                        mybir.AluOpType.mult, mybir.AluOpType.add)
                    nc.scalar.activation(st[:], st[:], AF.Sqrt)
                    nc.vector.reciprocal_approx_fast(out=st[:], in_=st[:])
                    bcr = rp.tile([P, N], FP32, tag="bcr", bufs=2,
                                  name=f"bcr{name}")
                    nc.gpsimd.partition_broadcast(bcr[:, :], st[:, :])
                    return bcr

                emit_rope("k", 0)
                emit_rope("k", 1)
                bcr_k = emit_rsqrt("k", 1)
                emit_rope("q", 0)
                emit_rope("q", 1)
                bcr_q = emit_rsqrt("q", 0)
                bcrs = {"k": bcr_k, "q": bcr_q}

                # ---- v projection (fills the collective-wait window) ----
                for tt in range(NJT):
                    acc = psv.tile([P, GF], FP32, tag="vacc")
                    for kt in range(KT):
                        nc.tensor.matmul(
                            acc[:],
                            xt[kt][:, tt * P:(tt + 1) * P],
                            wt[kt][:, 2 * GF:3 * GF],
                            start=(kt == 0),
                            stop=(kt == KT - 1),
                        )
                    out_ap = vbig[tt // 4][:].rearrange(
                        "p (g c) -> p g c",
                        g=16)[:, 4 * (tt % 4):4 * (tt % 4) + 4, 0:64]
                    in_ap = acc[:].rearrange("p (h d) -> p h d", h=4)
                    nc.scalar.activation(out_ap, in_ap, AF.Identity)

                # post-collective: fold r into the finished (unscaled) hats
                # with one in-place mul each; k first (its CC lands first)
                for name in ("k", "q"):
                    for mt in range(2):
                        hat = hats[(name, mt)]
                        nc.vector.tensor_mul(hat[:], hat[:], bcrs[name][:])

            # ---------------- phase 2: attention ----------------
            with (
                tc.tile_pool(name="ptp", bufs=4) as ptp,
                tc.tile_pool(name="onrm", bufs=2) as onrm,
                tc.tile_pool(name="outp", bufs=2) as outp,
                tc.tile_pool(name="tiny", bufs=2) as tiny,
                tc.tile_pool(name="psS", bufs=2, space="PSUM") as psS,
                tc.tile_pool(name="psO", bufs=4, space="PSUM") as psO,
            ):
                osb_pend = {}
                proj_pending = []

                def emit_proj_unit(pr, tt, half):
                    pss = psO.tile([P, 512], FP32, tag="O",
                                   name=f"pj{pr}_{tt}_{half}")
                    nc.tensor.matmul(
                        pss[:],
                        obig[pr][:, tt * P:(tt + 1) * P],
                        wout[pr][:, half * 512:(half + 1) * 512],
                        start=True, stop=True,
                    )
                    if (tt % 2, half) == (0, 0):
                        osb_pend[pr] = outp.tile([P, 2 * DIN], BF16,
                                                 tag="osb",
                                                 name=f"osb{pr}_{tt}")
                    osb = osb_pend[pr]
                    base = (tt % 2) * DIN + half * 512
                    nc.vector.tensor_copy(osb[:, base:base + 512], pss[:])
                    if (tt % 2, half) == (1, 1):
                        # gpsimd queue: keeps the bulky output DMAs from
                        # head-of-line-blocking the sync queue's small hops
                        nc.gpsimd.dma_start(
                            part_o[pr, (tt - 1) * P:(tt + 1) * P, :]
                            .rearrange("(a p) d -> p a d", p=P),
                            osb[:].rearrange("p (a d) -> p a d", a=2))

                def finish_block(pr, ib, o_ps):
                    # row 64 of each o_ps is the softmax denominator.
                    rrd, rr0, bc = [], [], []
                    for sub in range(2):
                        t = tiny.tile([65, IBW], FP32, tag="rrd",
                                      name=f"rrd{pr}_{ib}_{sub}")
                        nc.vector.tensor_copy(t[64:65, :],
                                              o_ps[sub][64:65, :])
                        rrd.append(t)
                    for sub in range(2):
                        t = tiny.tile([1, IBW], FP32, tag="rr0",
                                      name=f"rr0{pr}_{ib}_{sub}")
                        nc.sync.dma_start(t[:, :], rrd[sub][64:65, :])
                        rr0.append(t)
                    for sub in range(2):
                        nc.vector.reciprocal_approx_fast(
                            out=rr0[sub][:, :], in_=rr0[sub][:, :])
                    for sub in range(2):
                        t = tiny.tile([64, IBW], FP32, tag="bc",
                                      name=f"bc{pr}_{ib}_{sub}")
                        nc.gpsimd.partition_broadcast(t[:, :], rr0[sub][:, :])
                        bc.append(t)
                    nc.vector.tensor_mul(
                        obig[pr][0:64, ib * IBW:(ib + 1) * IBW],
                        o_ps[0][0:64, :], bc[0][:, :])
                    onr = onrm.tile([64, IBW], BF16, tag="onr")
                    nc.vector.tensor_mul(onr[:, :], o_ps[1][0:64, :],
                                         bc[1][:, :])
                    nc.sync.dma_start(
                        obig[pr][64:128, ib * IBW:(ib + 1) * IBW], onr[:, :])
                    # queue this block's out-projection into the stream
                    for tt in range(ib * 4, ib * 4 + 4):
                        for half in range(2):
                            proj_pending.append((pr, tt, half))

                steps = [(pr, ib, jt) for pr in range(2) for ib in range(NIB)
                         for jt in range(NJT)]
                p_sbs = {}
                o_ps_map = {}

                def emit_s(step):
                    pr, ib, jt = step
                    s_ps = psS.tile([P, 2 * IBW], FP32, tag="S")
                    for sub in range(2):
                        nc.tensor.matmul(
                            s_ps[:, sub * IBW:(sub + 1) * IBW],
                            hats[("k", pr)][sub * 64:(sub + 1) * 64,
                                            jt * P:(jt + 1) * P],
                            hats[("q", pr)][sub * 64:(sub + 1) * 64,
                                            ib * IBW:(ib + 1) * IBW],
                            start=True, stop=True,
                            tile_position=(64 * sub, 0),
                        )
                    p_sb = ptp.tile([P, 2 * IBW], BF16, tag="P",
                                    name=f"p{pr}_{ib}_{jt}")
                    nc.scalar.activation(p_sb[:, :], s_ps[:, :],
                                         AF.Exp, scale=0.125)
                    p_sbs[step] = p_sb

                def emit_av(step):
                    # AV lags one step behind S/exp so the in-order PE never
                    # waits on the current step's exp
                    pr, ib, jt = step
                    if (pr, ib) not in o_ps_map:
                        o_ps_map[(pr, ib)] = [
                            psO.tile([65, IBW], FP32, tag="O",
                                     name=f"o{pr}_{ib}_{s}")
                            for s in range(2)]
                    o_ps = o_ps_map[(pr, ib)]
                    p_sb = p_sbs.pop(step)
                    for sub in range(2):
                        h = 2 * pr + sub
                        nc.tensor.matmul(
                            o_ps[sub][:, :],
                            vt_slice(jt, h),
                            p_sb[:, sub * IBW:(sub + 1) * IBW],
                            start=(jt == 0), stop=(jt == NJT - 1),
                        )
                    if jt == NJT - 1:
                        finish_block(pr, ib, o_ps)

                emit_s(steps[0])
                for si, step in enumerate(steps):
                    if si + 1 < len(steps):
                        emit_s(steps[si + 1])
                    if si >= 1:
                        emit_av(steps[si - 1])
                    if proj_pending and si % 2 == 1:
                        emit_proj_unit(*proj_pending.pop(0))
                emit_av(steps[-1])
                while proj_pending:
                    emit_proj_unit(*proj_pending.pop(0))

    nc.compile()
    return nc


# ------------------------------------------------------------------- driver

def _rope_tables():
    half = DH // 2
    inv_freq = 1.0 / (ROPE_BASE ** (np.arange(half, dtype=np.float64) * 2.0
                                    / DH))
    freqs = np.arange(N, dtype=np.float64)[:, None] * inv_freq[None, :]
    cos = np.cos(freqs).T          # (32, N)
    sin = np.sin(freqs).T
    cos64 = np.concatenate([cos, cos], 0)            # (64, N)
    sin64 = np.concatenate([-sin, sin], 0)           # signed for rotate_half
    cos_t = np.ascontiguousarray(
        np.concatenate([cos64, cos64], 0).astype(BF16NP))  # (128, N)
    sin_t = np.ascontiguousarray(
        np.concatenate([sin64, sin64], 0).astype(BF16NP))
    return cos_t, sin_t


def kernel(input, w_qkv, b_qkv, q_scale, k_scale, w_out, b_out):
    trace = bool(os.environ.get("KERNEL_TRACE"))
    if "fused" not in _cache:
        _cache["fused"] = _build_fused()

    x = np.asarray(input, dtype=np.float32)
    w_qkv = np.asarray(w_qkv, dtype=np.float32)
    b_qkv = np.asarray(b_qkv, dtype=np.float32)
    qs = np.asarray(q_scale, dtype=np.float32)
    ks = np.asarray(k_scale, dtype=np.float32)
    w_out = np.asarray(w_out, dtype=np.float32)
    b_out = np.asarray(b_out, dtype=np.float32)

    wq = w_qkv[:, :DQ] * qs[None, :]
    wk = w_qkv[:, DQ:2 * DQ] * ks[None, :]
    wv = w_qkv[:, 2 * DQ:]
    bq = b_qkv[:DQ] * qs
    bk = b_qkv[DQ:2 * DQ] * ks
    bv = b_qkv[2 * DQ:]

    cos_t, sin_t = _rope_tables()
    xT = [np.ascontiguousarray(x[b].T.astype(BF16NP)) for b in range(B)]

    def col4(vec256_a, vec256_b):
        # -> (128, 4): [a_mt0 | a_mt1 | b_mt0 | b_mt1]
        return np.ascontiguousarray(np.stack(
            [vec256_a[:P], vec256_a[P:], vec256_b[:P], vec256_b[P:]],
            axis=1).astype(np.float32))

    ins = []
    for c in range(NCORES):
        b, g = divmod(c, NGROUP)
        sl = slice(g * GF, (g + 1) * GF)
        wcat = np.ascontiguousarray(np.concatenate(
            [wq[:, sl], wk[:, sl], wv[:, sl]], axis=1).astype(BF16NP))
        ins.append({
            "xT": xT[b],
            "wcat": wcat,
            "bqk": col4(bq[sl], bk[sl]),
            "winv": np.ascontiguousarray(np.repeat(
                col4(1.0 / np.square(qs[sl]), 1.0 / np.square(ks[sl])),
                32, axis=1).astype(BF16NP)),
            "cos_t": cos_t,
            "sin_t": sin_t,
            "wout": np.ascontiguousarray(w_out[sl, :].astype(BF16NP)),
        })

    r = run_bass_kernel_spmd(_cache["fused"], ins,
                             core_ids=list(range(NCORES)), trace=trace)
    if trace:
        LAST_EXEC_NS["fused"] = r.exec_time_ns
        LAST_RESULTS["fused"] = r

    base = (bv.astype(np.float64) @ w_out.astype(np.float64)
            + b_out.astype(np.float64))
    out = np.zeros((B, N, DIN), np.float32)
    for b in range(B):
        acc = np.zeros((N, DIN), np.float64)
        for g in range(NGROUP):
            p = r.results[NGROUP * b + g]["part"].astype(np.float64)
            acc += p[0]
            acc += p[1]
        out[b] = (acc + base[None, :]).astype(np.float32)
    return out
